# revision 2
# baseline (speedup 1.0000x reference)
"""Trainium2 Bass kernel for ConditionalAttentionFusion-v2.

Math (per batch b, channel c, pixel y,x):
    CD   = concat(rgb_var, d_var)                       # [2,H,W], shared
    AB   = Wp[c,0]*rgb + Wp[c,1]*d
    CDc  = conv3x3(CD, W_unc[c])                        # 2-in 1-out per channel
    G    = Wt[c,0]*AB + Wt[c,1]*CDc
    out  = rgb*G + d*(1-G) = d + (rgb-d)*G

Strategy: pure data parallel over 8 cores (core = (batch, H-half), slab of 256
rows).  On each core the 3x3 conv (y-taps) + per-channel 1x1 terms are computed
on the TensorEngine as banded/diagonal-matrix matmuls accumulating into PSUM:

    G[r, x] = sum_{i,kx} Band[c,i,kx].T @ V_i[:, x+kx]    (6 band matmuls)
            + diag(a0[c]).T @ rgb + diag(a1[c]).T @ d     (2 diag matmuls)

where Band[p=r+ky, m=r] = Wt[c,1]*W_unc[c,i,ky,kx] folds the three ky taps of
the conv into one matmul (output rows 0..125 valid per 128-row V tile).  The
x-shifts (kx) are free-dim offsets into an x-padded V tile; the y-halo is
handled host-side by padding the var slab.  VectorE then does the 3-op tail:
diff = rgb-d; P = diff*G(PSUM); out = P + d.

A slab of 256 rows = two 126-row band tiles + a 4-row remainder.  The
remainder stacks all 19 channels into one matmul group (output partition
m = 4c+r), so it costs only 6 band + 2 diag matmuls total.

All band/diag matrices are precomputed host-side in numpy from the runtime
weight tensors and passed as extra kernel inputs.
"""
import sys

if "/opt/trn_rl_repo" not in sys.path:
    sys.path.insert(0, "/opt/trn_rl_repo")

import numpy as np

import concourse.bacc as bacc
import concourse.mybir as mybir
import concourse.tile as tile
from concourse.bass_utils import run_bass_kernel_spmd

F32 = mybir.dt.float32
B, C, H, W = 4, 19, 512, 1024
R = 256              # slab rows per core
NCORES = 8
MAIN_Y0 = (0, 126)   # 126-row band tiles
REM_Y0 = 252         # 4-row remainder, stacked over channels


# ----------------------------------------------------------------- host math
def _build_mats(W_prob, W_unc, W_total):
    a0 = W_total[:, 0] * W_prob[:, 0]
    a1 = W_total[:, 0] * W_prob[:, 1]
    Wp = W_total[:, 1][:, None, None, None] * W_unc          # [C,2,3,3]

    bands = np.zeros((C, 128, 6, 128), np.float32)           # [c,p,s,m]
    r = np.arange(126)
    for i in range(2):
        for kx in range(3):
            s = i * 3 + kx
            for ky in range(3):
                bands[:, r + ky, s, r] = Wp[:, i, ky, kx][:, None]

    diags = np.zeros((C, 128, 2, 128), np.float32)           # [c,p,j,m]
    m = np.arange(128)
    diags[:, m, 0, m] = a0[:, None]
    diags[:, m, 1, m] = a1[:, None]

    remb = np.zeros((6, 6, 128), np.float32)                 # [p,s,m], m=4c+r
    rr = np.arange(4)
    for i in range(2):
        for kx in range(3):
            s = i * 3 + kx
            for ky in range(3):
                for c in range(C):
                    remb[rr + ky, s, 4 * c + rr] = Wp[c, i, ky, kx]

    remd = np.zeros((76, 2, 76), np.float32)                 # [p,j,m], p=m=4c+r
    p = np.arange(76)
    remd[p, 0, p] = np.repeat(a0, 4)
    remd[p, 1, p] = np.repeat(a1, 4)

    return (bands.reshape(C, 128, 768), diags.reshape(C, 128, 256),
            remb.reshape(6, 768), remd.reshape(76, 152))


# ------------------------------------------------------------- bass program
_CACHE = {}


def _build_program():
    nc = bacc.Bacc("TRN2", debug=False, num_devices=NCORES)
    f = F32
    rgb_s = nc.dram_tensor("rgb_s", [C, R, W], f, kind="ExternalInput").ap()
    d_s = nc.dram_tensor("d_s", [C, R, W], f, kind="ExternalInput").ap()
    var_s = nc.dram_tensor("var_s", [2, R + 2, W + 2], f, kind="ExternalInput").ap()
    bands = nc.dram_tensor("bands", [C, 128, 768], f, kind="ExternalInput").ap()
    diags = nc.dram_tensor("diags", [C, 128, 256], f, kind="ExternalInput").ap()
    remb = nc.dram_tensor("remb", [6, 768], f, kind="ExternalInput").ap()
    remd = nc.dram_tensor("remd", [76, 152], f, kind="ExternalInput").ap()
    out_s = nc.dram_tensor("out_s", [C, R, W], f, kind="ExternalOutput").ap()

    with tile.TileContext(nc) as tc:
        with (
            tc.tile_pool(name="wpool", bufs=1) as wpool,
            tc.tile_pool(name="vpool", bufs=4) as vpool,
            tc.tile_pool(name="io", bufs=3) as io,
            tc.tile_pool(name="tmp", bufs=2) as tmp,
            tc.tile_pool(name="psum", bufs=3, space="PSUM") as psum,
        ):
            band_sb, diag_sb = [], []
            for c in range(C):
                bt = wpool.tile([128, 768], f, tag=f"band{c}", name=f"band{c}")
                nc.sync.dma_start(out=bt[:], in_=bands[c])
                dt_ = wpool.tile([128, 256], f, tag=f"diag{c}", name=f"diag{c}")
                nc.sync.dma_start(out=dt_[:], in_=diags[c])
                band_sb.append(bt)
                diag_sb.append(dt_)
            remb_sb = wpool.tile([6, 768], f, tag="remb", name="remb_sb")
            nc.sync.dma_start(out=remb_sb[:], in_=remb[:])
            remd_sb = wpool.tile([76, 152], f, tag="remd", name="remd_sb")
            nc.sync.dma_start(out=remd_sb[:], in_=remd[:])

            # ---------------- main 126-row band tiles
            for y0 in MAIN_Y0:
                vt = []
                for i in range(2):
                    v = vpool.tile([128, W + 2], f, tag="v", name=f"v{i}_{y0}")
                    nc.sync.dma_start(out=v[:], in_=var_s[i, y0:y0 + 128, :])
                    vt.append(v)
                for c in range(C):
                    rt = io.tile([126, W], f, tag="r", name=f"r{y0}_{c}")
                    nc.sync.dma_start(out=rt[:], in_=rgb_s[c, y0:y0 + 126, :])
                    dt = io.tile([126, W], f, tag="d", name=f"d{y0}_{c}")
                    nc.sync.dma_start(out=dt[:], in_=d_s[c, y0:y0 + 126, :])

                    ps = psum.tile([128, W], f, tag="ps", name=f"ps{y0}_{c}")
                    for xb in (0, 512):
                        for s in range(6):
                            i, kx = divmod(s, 3)
                            nc.tensor.matmul(
                                ps[:, xb:xb + 512],
                                band_sb[c][:, s * 128:(s + 1) * 128],
                                vt[i][:, xb + kx:xb + kx + 512],
                                start=(s == 0), stop=False)
                        nc.tensor.matmul(
                            ps[:126, xb:xb + 512],
                            diag_sb[c][:126, 0:126],
                            rt[:, xb:xb + 512], start=False, stop=False)
                        nc.tensor.matmul(
                            ps[:126, xb:xb + 512],
                            diag_sb[c][:126, 128:254],
                            dt[:, xb:xb + 512], start=False, stop=True)

                    diff = tmp.tile([126, W], f, tag="diff", name=f"diff{y0}_{c}")
                    nc.vector.tensor_sub(out=diff[:], in0=rt[:], in1=dt[:])
                    prod = tmp.tile([126, W], f, tag="prod", name=f"prod{y0}_{c}")
                    nc.vector.tensor_mul(out=prod[:], in0=diff[:], in1=ps[:126, :])
                    ot = io.tile([126, W], f, tag="o", name=f"o{y0}_{c}")
                    nc.vector.tensor_add(out=ot[:], in0=prod[:], in1=dt[:])
                    nc.sync.dma_start(out=out_s[c, y0:y0 + 126, :], in_=ot[:])

            # ---------------- 4-row remainder, all channels stacked (m = 4c+r)
            vr = []
            for i in range(2):
                v = vpool.tile([6, W + 2], f, tag=f"vrem{i}", name=f"vrem{i}", bufs=1)
                nc.sync.dma_start(out=v[:], in_=var_s[i, REM_Y0:REM_Y0 + 6, :])
                vr.append(v)
            rr = io.tile([76, W], f, tag="rrem", name="rrem", bufs=1)
            dr = io.tile([76, W], f, tag="drem", name="drem", bufs=1)
            for c in range(C):
                nc.sync.dma_start(out=rr[4 * c:4 * c + 4, :],
                                  in_=rgb_s[c, REM_Y0:REM_Y0 + 4, :])
                nc.sync.dma_start(out=dr[4 * c:4 * c + 4, :],
                                  in_=d_s[c, REM_Y0:REM_Y0 + 4, :])
            ps = psum.tile([128, W], f, tag="ps", name="ps_rem")
            for xb in (0, 512):
                for s in range(6):
                    i, kx = divmod(s, 3)
                    nc.tensor.matmul(
                        ps[:, xb:xb + 512],
                        remb_sb[:, s * 128:(s + 1) * 128],
                        vr[i][:, xb + kx:xb + kx + 512],
                        start=(s == 0), stop=False)
                nc.tensor.matmul(ps[:76, xb:xb + 512], remd_sb[:, 0:76],
                                 rr[:, xb:xb + 512], start=False, stop=False)
                nc.tensor.matmul(ps[:76, xb:xb + 512], remd_sb[:, 76:152],
                                 dr[:, xb:xb + 512], start=False, stop=True)
            diff = tmp.tile([76, W], f, tag="diffrem", name="diff_rem", bufs=1)
            nc.vector.tensor_sub(out=diff[:], in0=rr[:], in1=dr[:])
            prod = tmp.tile([76, W], f, tag="prodrem", name="prod_rem", bufs=1)
            nc.vector.tensor_mul(out=prod[:], in0=diff[:], in1=ps[:76, :])
            ot = io.tile([76, W], f, tag="orem", name="o_rem", bufs=1)
            nc.vector.tensor_add(out=ot[:], in0=prod[:], in1=dr[:])
            for c in range(C):
                nc.sync.dma_start(out=out_s[c, REM_Y0:REM_Y0 + 4, :],
                                  in_=ot[4 * c:4 * c + 4, :])

    nc.compile()
    return nc


def _shard_inputs(rgb, d, rgb_var, d_var, W_prob, W_unc, W_total):
    bands, diags, remb, remd = _build_mats(
        np.asarray(W_prob, np.float32),
        np.asarray(W_unc, np.float32),
        np.asarray(W_total, np.float32))
    in_maps = []
    for core in range(NCORES):
        b, half = divmod(core, 2)
        h0 = half * R
        var = np.zeros((2, R + 2, W + 2), np.float32)
        lo, hi = max(h0 - 1, 0), min(h0 + R + 1, H)
        var[0, lo - h0 + 1:hi - h0 + 1, 1:W + 1] = rgb_var[b, 0, lo:hi, :]
        var[1, lo - h0 + 1:hi - h0 + 1, 1:W + 1] = d_var[b, 0, lo:hi, :]
        in_maps.append({
            "rgb_s": np.ascontiguousarray(rgb[b, :, h0:h0 + R, :], np.float32),
            "d_s": np.ascontiguousarray(d[b, :, h0:h0 + R, :], np.float32),
            "var_s": var,
            "bands": bands, "diags": diags, "remb": remb, "remd": remd,
        })
    return in_maps


def run(trace=False, **inputs):
    if "nc" not in _CACHE:
        _CACHE["nc"] = _build_program()
    nc = _CACHE["nc"]
    in_maps = _shard_inputs(**inputs)
    res = run_bass_kernel_spmd(nc, in_maps, list(range(NCORES)), trace=trace)
    out = np.empty((B, C, H, W), np.float32)
    for core in range(NCORES):
        b, half = divmod(core, 2)
        out[b, :, half * R:(half + 1) * R, :] = res.results[core]["out_s"]
    return out, res


def kernel(**inputs):
    out, _ = run(trace=False, **inputs)
    return out


# revision 3
# speedup vs baseline: 1.7543x; 1.7543x over previous
"""Trainium2 Bass kernel for ConditionalAttentionFusion-v2.

Math (per batch b, channel c, pixel y,x):
    CD   = concat(rgb_var, d_var)                       # [2,H,W], shared
    AB   = Wp[c,0]*rgb + Wp[c,1]*d
    CDc  = conv3x3(CD, W_unc[c])                        # 2-in 1-out per channel
    G    = Wt[c,0]*AB + Wt[c,1]*CDc
    out  = rgb*G + d*(1-G) = d + (rgb-d)*G

Strategy: pure data parallel over 8 cores (core = (batch, H-half), slab of 256
rows).  On each core the 3x3 conv (y-taps) + per-channel 1x1 terms are computed
on the TensorEngine as banded/diagonal-matrix matmuls accumulating into PSUM:

    G[r, x] = sum_{i,kx} Band[c,i,kx].T @ V_i[:, x+kx]    (6 band matmuls)
            + diag(a0[c]).T @ rgb + diag(a1[c]).T @ d     (2 diag matmuls)

where Band[p=r+ky, m=r] = Wt[c,1]*W_unc[c,i,ky,kx] folds the three ky taps of
the conv into one matmul (output rows 0..125 valid per 128-row V tile).  The
x-shifts (kx) are free-dim offsets into an x-padded V tile; the y-halo is
handled host-side by padding the var slab.  VectorE then does the 3-op tail:
diff = rgb-d; P = diff*G(PSUM); out = P + d.

A slab of 256 rows = two 126-row band tiles + a 4-row remainder.  The
remainder stacks all 19 channels into one matmul group (output partition
m = 4c+r), so it costs only 6 band + 2 diag matmuls total.

All band/diag matrices are precomputed host-side in numpy from the runtime
weight tensors and passed as extra kernel inputs.
"""
import sys

if "/opt/trn_rl_repo" not in sys.path:
    sys.path.insert(0, "/opt/trn_rl_repo")

import numpy as np

import concourse.bacc as bacc
import concourse.mybir as mybir
import concourse.tile as tile
from concourse.bass_utils import run_bass_kernel_spmd

F32 = mybir.dt.float32
F32R = mybir.dt.float32r
B, C, H, W = 4, 19, 512, 1024
R = 256              # slab rows per core
NCORES = 8
MAIN_Y0 = (0, 126)   # 126-row band tiles
REM_Y0 = 252         # 4-row remainder, stacked over channels


# ----------------------------------------------------------------- host math
def _build_mats(W_prob, W_unc, W_total):
    a0 = W_total[:, 0] * W_prob[:, 0]
    a1 = W_total[:, 0] * W_prob[:, 1]
    Wp = W_total[:, 1][:, None, None, None] * W_unc          # [C,2,3,3]

    bands = np.zeros((C, 128, 6, 128), np.float32)           # [c,p,s,m]
    r = np.arange(126)
    for i in range(2):
        for kx in range(3):
            s = i * 3 + kx
            for ky in range(3):
                bands[:, r + ky, s, r] = Wp[:, i, ky, kx][:, None]

    diags = np.zeros((C, 128, 2, 128), np.float32)           # [c,p,j,m]
    m = np.arange(128)
    diags[:, m, 0, m] = a0[:, None]
    diags[:, m, 1, m] = a1[:, None]

    remb = np.zeros((6, 6, 128), np.float32)                 # [p,s,m], m=4c+r
    rr = np.arange(4)
    for i in range(2):
        for kx in range(3):
            s = i * 3 + kx
            for ky in range(3):
                for c in range(C):
                    remb[rr + ky, s, 4 * c + rr] = Wp[c, i, ky, kx]

    remd = np.zeros((76, 2, 76), np.float32)                 # [p,j,m], p=m=4c+r
    p = np.arange(76)
    remd[p, 0, p] = np.repeat(a0, 4)
    remd[p, 1, p] = np.repeat(a1, 4)

    return (bands.reshape(C, 128, 768), diags.reshape(C, 128, 256),
            remb.reshape(6, 768), remd.reshape(76, 152))


# ------------------------------------------------------------- bass program
_CACHE = {}


def _build_program():
    nc = bacc.Bacc("TRN2", debug=False, num_devices=NCORES)
    f = F32R
    rgb_s = nc.dram_tensor("rgb_s", [C, R, W], f, kind="ExternalInput").ap()
    d_s = nc.dram_tensor("d_s", [C, R, W], f, kind="ExternalInput").ap()
    var_s = nc.dram_tensor("var_s", [2, R + 2, W + 2], f, kind="ExternalInput").ap()
    bands = nc.dram_tensor("bands", [C, 128, 768], f, kind="ExternalInput").ap()
    diags = nc.dram_tensor("diags", [C, 128, 256], f, kind="ExternalInput").ap()
    remb = nc.dram_tensor("remb", [6, 768], f, kind="ExternalInput").ap()
    remd = nc.dram_tensor("remd", [76, 152], f, kind="ExternalInput").ap()
    out_s = nc.dram_tensor("out_s", [C, R, W], f, kind="ExternalOutput").ap()

    with tile.TileContext(nc) as tc:
        with (
            tc.tile_pool(name="wpool", bufs=1) as wpool,
            tc.tile_pool(name="vpool", bufs=4) as vpool,
            tc.tile_pool(name="io", bufs=3) as io,
            tc.tile_pool(name="tmp", bufs=2) as tmp,
            tc.tile_pool(name="psum", bufs=4, space="PSUM") as psum,
        ):
            band_sb, diag_sb = [], []
            for c in range(C):
                bt = wpool.tile([128, 768], f, tag=f"band{c}", name=f"band{c}")
                nc.sync.dma_start(out=bt[:], in_=bands[c])
                dt_ = wpool.tile([128, 256], f, tag=f"diag{c}", name=f"diag{c}")
                nc.sync.dma_start(out=dt_[:], in_=diags[c])
                band_sb.append(bt)
                diag_sb.append(dt_)
            remb_sb = wpool.tile([6, 768], f, tag="remb", name="remb_sb")
            nc.sync.dma_start(out=remb_sb[:], in_=remb[:])
            remd_sb = wpool.tile([76, 152], f, tag="remd", name="remd_sb")
            nc.sync.dma_start(out=remd_sb[:], in_=remd[:])

            # ---------------- main 126-row band tiles
            for y0 in MAIN_Y0:
                vt = []
                for i in range(2):
                    v = vpool.tile([128, W + 2], f, tag="v", name=f"v{i}_{y0}")
                    nc.sync.dma_start(out=v[:], in_=var_s[i, y0:y0 + 128, :])
                    vt.append(v)
                for c in range(C):
                    rt = io.tile([126, W], f, tag="r", name=f"r{y0}_{c}")
                    nc.sync.dma_start(out=rt[:], in_=rgb_s[c, y0:y0 + 126, :])
                    dt = io.tile([126, W], f, tag="d", name=f"d{y0}_{c}")
                    nc.sync.dma_start(out=dt[:], in_=d_s[c, y0:y0 + 126, :])

                    ps = psum.tile([128, W], F32, tag="ps", name=f"ps{y0}_{c}")
                    for xb in (0, 512):
                        for s in range(6):
                            i, kx = divmod(s, 3)
                            nc.tensor.matmul(
                                ps[:, xb:xb + 512],
                                band_sb[c][:, s * 128:(s + 1) * 128],
                                vt[i][:, xb + kx:xb + kx + 512],
                                start=(s == 0), stop=False)
                        nc.tensor.matmul(
                            ps[:126, xb:xb + 512],
                            diag_sb[c][:126, 0:126],
                            rt[:, xb:xb + 512], start=False, stop=False)
                        nc.tensor.matmul(
                            ps[:126, xb:xb + 512],
                            diag_sb[c][:126, 128:254],
                            dt[:, xb:xb + 512], start=False, stop=True)

                    diff = tmp.tile([126, W], f, tag="diff", name=f"diff{y0}_{c}")
                    nc.vector.tensor_sub(out=diff[:], in0=rt[:], in1=dt[:])
                    prod = tmp.tile([126, W], f, tag="prod", name=f"prod{y0}_{c}")
                    nc.vector.tensor_mul(out=prod[:], in0=diff[:], in1=ps[:126, :])
                    ot = io.tile([126, W], f, tag="o", name=f"o{y0}_{c}")
                    nc.vector.tensor_add(out=ot[:], in0=prod[:], in1=dt[:])
                    nc.sync.dma_start(out=out_s[c, y0:y0 + 126, :], in_=ot[:])

            # ---------------- 4-row remainder, all channels stacked (m = 4c+r)
            vr = []
            for i in range(2):
                v = vpool.tile([6, W + 2], f, tag=f"vrem{i}", name=f"vrem{i}", bufs=1)
                nc.sync.dma_start(out=v[:], in_=var_s[i, REM_Y0:REM_Y0 + 6, :])
                vr.append(v)
            rr = io.tile([76, W], f, tag="rrem", name="rrem", bufs=1)
            dr = io.tile([76, W], f, tag="drem", name="drem", bufs=1)
            for c in range(C):
                nc.sync.dma_start(out=rr[4 * c:4 * c + 4, :],
                                  in_=rgb_s[c, REM_Y0:REM_Y0 + 4, :])
                nc.sync.dma_start(out=dr[4 * c:4 * c + 4, :],
                                  in_=d_s[c, REM_Y0:REM_Y0 + 4, :])
            ps = psum.tile([128, W], F32, tag="ps", name="ps_rem")
            for xb in (0, 512):
                for s in range(6):
                    i, kx = divmod(s, 3)
                    nc.tensor.matmul(
                        ps[:, xb:xb + 512],
                        remb_sb[:, s * 128:(s + 1) * 128],
                        vr[i][:, xb + kx:xb + kx + 512],
                        start=(s == 0), stop=False)
                nc.tensor.matmul(ps[:76, xb:xb + 512], remd_sb[:, 0:76],
                                 rr[:, xb:xb + 512], start=False, stop=False)
                nc.tensor.matmul(ps[:76, xb:xb + 512], remd_sb[:, 76:152],
                                 dr[:, xb:xb + 512], start=False, stop=True)
            diff = tmp.tile([76, W], f, tag="diffrem", name="diff_rem", bufs=1)
            nc.vector.tensor_sub(out=diff[:], in0=rr[:], in1=dr[:])
            prod = tmp.tile([76, W], f, tag="prodrem", name="prod_rem", bufs=1)
            nc.vector.tensor_mul(out=prod[:], in0=diff[:], in1=ps[:76, :])
            ot = io.tile([76, W], f, tag="orem", name="o_rem", bufs=1)
            nc.vector.tensor_add(out=ot[:], in0=prod[:], in1=dr[:])
            for c in range(C):
                nc.sync.dma_start(out=out_s[c, REM_Y0:REM_Y0 + 4, :],
                                  in_=ot[4 * c:4 * c + 4, :])

    nc.compile()
    return nc


def _shard_inputs(rgb, d, rgb_var, d_var, W_prob, W_unc, W_total):
    bands, diags, remb, remd = _build_mats(
        np.asarray(W_prob, np.float32),
        np.asarray(W_unc, np.float32),
        np.asarray(W_total, np.float32))
    in_maps = []
    for core in range(NCORES):
        b, half = divmod(core, 2)
        h0 = half * R
        var = np.zeros((2, R + 2, W + 2), np.float32)
        lo, hi = max(h0 - 1, 0), min(h0 + R + 1, H)
        var[0, lo - h0 + 1:hi - h0 + 1, 1:W + 1] = rgb_var[b, 0, lo:hi, :]
        var[1, lo - h0 + 1:hi - h0 + 1, 1:W + 1] = d_var[b, 0, lo:hi, :]
        in_maps.append({
            "rgb_s": np.ascontiguousarray(rgb[b, :, h0:h0 + R, :], np.float32),
            "d_s": np.ascontiguousarray(d[b, :, h0:h0 + R, :], np.float32),
            "var_s": var,
            "bands": bands, "diags": diags, "remb": remb, "remd": remd,
        })
    return in_maps


def run(trace=False, **inputs):
    if "nc" not in _CACHE:
        _CACHE["nc"] = _build_program()
    nc = _CACHE["nc"]
    in_maps = _shard_inputs(**inputs)
    res = run_bass_kernel_spmd(nc, in_maps, list(range(NCORES)), trace=trace)
    out = np.empty((B, C, H, W), np.float32)
    for core in range(NCORES):
        b, half = divmod(core, 2)
        out[b, :, half * R:(half + 1) * R, :] = res.results[core]["out_s"]
    return out, res


def kernel(**inputs):
    out, _ = run(trace=False, **inputs)
    return out


# revision 4
# speedup vs baseline: 1.8608x; 1.0607x over previous
"""Trainium2 Bass kernel for ConditionalAttentionFusion-v2.

Math (per batch b, channel c, pixel y,x):
    CD   = concat(rgb_var, d_var)                       # [2,H,W], shared
    AB   = Wp[c,0]*rgb + Wp[c,1]*d
    CDc  = conv3x3(CD, W_unc[c])                        # 2-in 1-out per channel
    G    = Wt[c,0]*AB + Wt[c,1]*CDc
    out  = rgb*G + d*(1-G) = d + (rgb-d)*G

Strategy: pure data parallel over 8 cores (core = (batch, H-half), slab of 256
rows).  On each core the 3x3 conv (y-taps) + per-channel 1x1 terms are computed
on the TensorEngine as banded/diagonal-matrix matmuls accumulating into PSUM:

    G[r, x] = sum_{i,kx} Band[c,i,kx].T @ V_i[:, x+kx]    (6 band matmuls)
            + diag(a0[c]).T @ rgb + diag(a1[c]).T @ d     (2 diag matmuls)

where Band[p=r+ky, m=r] = Wt[c,1]*W_unc[c,i,ky,kx] folds the three ky taps of
the conv into one matmul (output rows 0..125 valid per 128-row V tile).  The
x-shifts (kx) are free-dim offsets into an x-padded V tile; the y-halo is
handled host-side by padding the var slab.  VectorE then does the 3-op tail:
diff = rgb-d; P = diff*G(PSUM); out = P + d.

A slab of 256 rows = two 126-row band tiles + a 4-row remainder.  The
remainder stacks all 19 channels into one matmul group (output partition
m = 4c+r), so it costs only 6 band + 2 diag matmuls total.

All band/diag matrices are precomputed host-side in numpy from the runtime
weight tensors and passed as extra kernel inputs.
"""
import sys

if "/opt/trn_rl_repo" not in sys.path:
    sys.path.insert(0, "/opt/trn_rl_repo")

import numpy as np

import concourse.bacc as bacc
import concourse.mybir as mybir
import concourse.tile as tile
from concourse.bass_utils import run_bass_kernel_spmd

F32 = mybir.dt.float32
F32R = mybir.dt.float32r
B, C, H, W = 4, 19, 512, 1024
R = 256              # slab rows per core
NCORES = 8
MAIN_Y0 = (0, 126)   # 126-row band tiles
REM_Y0 = 252         # 4-row remainder, stacked over channels


# ----------------------------------------------------------------- host math
def _build_mats(W_prob, W_unc, W_total):
    a0 = W_total[:, 0] * W_prob[:, 0]
    a1 = W_total[:, 0] * W_prob[:, 1]
    Wp = W_total[:, 1][:, None, None, None] * W_unc          # [C,2,3,3]

    bands = np.zeros((C, 128, 6, 128), np.float32)           # [c,p,s,m]
    r = np.arange(126)
    for i in range(2):
        for kx in range(3):
            s = i * 3 + kx
            for ky in range(3):
                bands[:, r + ky, s, r] = Wp[:, i, ky, kx][:, None]

    diags = np.zeros((C, 128, 2, 128), np.float32)           # [c,p,j,m]
    m = np.arange(128)
    diags[:, m, 0, m] = a0[:, None]
    diags[:, m, 1, m] = a1[:, None]

    remb = np.zeros((6, 6, 128), np.float32)                 # [p,s,m], m=4c+r
    rr = np.arange(4)
    for i in range(2):
        for kx in range(3):
            s = i * 3 + kx
            for ky in range(3):
                for c in range(C):
                    remb[rr + ky, s, 4 * c + rr] = Wp[c, i, ky, kx]

    remd = np.zeros((76, 2, 76), np.float32)                 # [p,j,m], p=m=4c+r
    p = np.arange(76)
    remd[p, 0, p] = np.repeat(a0, 4)
    remd[p, 1, p] = np.repeat(a1, 4)

    return (bands.reshape(C, 128, 768), diags.reshape(C, 128, 256),
            remb.reshape(6, 768), remd.reshape(76, 152))


# ------------------------------------------------------------- bass program
_CACHE = {}


def _build_program():
    nc = bacc.Bacc("TRN2", debug=False, num_devices=NCORES)
    f = F32R
    rgb_s = nc.dram_tensor("rgb_s", [C, R, W], F32, kind="ExternalInput").ap()
    d_s = nc.dram_tensor("d_s", [C, R, W], F32, kind="ExternalInput").ap()
    var_s = nc.dram_tensor("var_s", [2, R + 2, W + 2], f, kind="ExternalInput").ap()
    bands = nc.dram_tensor("bands", [C, 128, 768], f, kind="ExternalInput").ap()
    diags = nc.dram_tensor("diags", [C, 128, 256], F32, kind="ExternalInput").ap()
    remb = nc.dram_tensor("remb", [6, 768], f, kind="ExternalInput").ap()
    remd = nc.dram_tensor("remd", [76, 152], F32, kind="ExternalInput").ap()
    out_s = nc.dram_tensor("out_s", [C, R, W], F32, kind="ExternalOutput").ap()

    with tile.TileContext(nc) as tc:
        with (
            tc.tile_pool(name="wpool", bufs=1) as wpool,
            tc.tile_pool(name="vpool", bufs=4) as vpool,
            tc.tile_pool(name="io", bufs=3) as io,
            tc.tile_pool(name="tmp", bufs=2) as tmp,
            tc.tile_pool(name="psum", bufs=4, space="PSUM") as psum,
        ):
            band_sb, diag_sb = [], []
            for c in range(C):
                bt = wpool.tile([128, 768], f, tag=f"band{c}", name=f"band{c}")
                nc.sync.dma_start(out=bt[:], in_=bands[c])
                dt_ = wpool.tile([128, 256], F32, tag=f"diag{c}", name=f"diag{c}")
                nc.sync.dma_start(out=dt_[:], in_=diags[c])
                band_sb.append(bt)
                diag_sb.append(dt_)
            remb_sb = wpool.tile([6, 768], f, tag="remb", name="remb_sb")
            nc.sync.dma_start(out=remb_sb[:], in_=remb[:])
            remd_sb = wpool.tile([76, 152], F32, tag="remd", name="remd_sb")
            nc.sync.dma_start(out=remd_sb[:], in_=remd[:])

            # ---------------- main 126-row band tiles
            for y0 in MAIN_Y0:
                vt = []
                for i in range(2):
                    v = vpool.tile([128, W + 2], f, tag="v", name=f"v{i}_{y0}")
                    nc.sync.dma_start(out=v[:], in_=var_s[i, y0:y0 + 128, :])
                    vt.append(v)
                for c in range(C):
                    rt = io.tile([126, W], F32, tag="r", name=f"r{y0}_{c}")
                    nc.sync.dma_start(out=rt[:], in_=rgb_s[c, y0:y0 + 126, :])
                    dt = io.tile([126, W], F32, tag="d", name=f"d{y0}_{c}")
                    nc.sync.dma_start(out=dt[:], in_=d_s[c, y0:y0 + 126, :])

                    ps = psum.tile([128, W], F32, tag="ps", name=f"ps{y0}_{c}")
                    for xb in (0, 512):
                        for s in range(6):
                            i, kx = divmod(s, 3)
                            nc.tensor.matmul(
                                ps[:, xb:xb + 512],
                                band_sb[c][:, s * 128:(s + 1) * 128],
                                vt[i][:, xb + kx:xb + kx + 512],
                                start=(s == 0), stop=False)
                        nc.tensor.matmul(
                            ps[:126, xb:xb + 512],
                            diag_sb[c][:126, 0:126],
                            rt[:, xb:xb + 512], start=False, stop=False)
                        nc.tensor.matmul(
                            ps[:126, xb:xb + 512],
                            diag_sb[c][:126, 128:254],
                            dt[:, xb:xb + 512], start=False, stop=True)

                    diff = tmp.tile([126, W], F32, tag="diff", name=f"diff{y0}_{c}")
                    nc.vector.tensor_sub(out=diff[:], in0=rt[:], in1=dt[:])
                    prod = tmp.tile([126, W], F32, tag="prod", name=f"prod{y0}_{c}")
                    nc.vector.tensor_mul(out=prod[:], in0=diff[:], in1=ps[:126, :])
                    ot = io.tile([126, W], F32, tag="o", name=f"o{y0}_{c}")
                    nc.vector.tensor_add(out=ot[:], in0=prod[:], in1=dt[:])
                    nc.sync.dma_start(out=out_s[c, y0:y0 + 126, :], in_=ot[:])

            # ---------------- 4-row remainder, all channels stacked (m = 4c+r)
            vr = []
            for i in range(2):
                v = vpool.tile([6, W + 2], f, tag=f"vrem{i}", name=f"vrem{i}", bufs=1)
                nc.sync.dma_start(out=v[:], in_=var_s[i, REM_Y0:REM_Y0 + 6, :])
                vr.append(v)
            rr = io.tile([76, W], F32, tag="rrem", name="rrem", bufs=1)
            dr = io.tile([76, W], F32, tag="drem", name="drem", bufs=1)
            for c in range(C):
                nc.sync.dma_start(out=rr[4 * c:4 * c + 4, :],
                                  in_=rgb_s[c, REM_Y0:REM_Y0 + 4, :])
                nc.sync.dma_start(out=dr[4 * c:4 * c + 4, :],
                                  in_=d_s[c, REM_Y0:REM_Y0 + 4, :])
            ps = psum.tile([128, W], F32, tag="ps", name="ps_rem")
            for xb in (0, 512):
                for s in range(6):
                    i, kx = divmod(s, 3)
                    nc.tensor.matmul(
                        ps[:, xb:xb + 512],
                        remb_sb[:, s * 128:(s + 1) * 128],
                        vr[i][:, xb + kx:xb + kx + 512],
                        start=(s == 0), stop=False)
                nc.tensor.matmul(ps[:76, xb:xb + 512], remd_sb[:, 0:76],
                                 rr[:, xb:xb + 512], start=False, stop=False)
                nc.tensor.matmul(ps[:76, xb:xb + 512], remd_sb[:, 76:152],
                                 dr[:, xb:xb + 512], start=False, stop=True)
            diff = tmp.tile([76, W], F32, tag="diffrem", name="diff_rem", bufs=1)
            nc.vector.tensor_sub(out=diff[:], in0=rr[:], in1=dr[:])
            prod = tmp.tile([76, W], F32, tag="prodrem", name="prod_rem", bufs=1)
            nc.vector.tensor_mul(out=prod[:], in0=diff[:], in1=ps[:76, :])
            ot = io.tile([76, W], F32, tag="orem", name="o_rem", bufs=1)
            nc.vector.tensor_add(out=ot[:], in0=prod[:], in1=dr[:])
            for c in range(C):
                nc.sync.dma_start(out=out_s[c, REM_Y0:REM_Y0 + 4, :],
                                  in_=ot[4 * c:4 * c + 4, :])

    nc.compile()
    return nc


def _shard_inputs(rgb, d, rgb_var, d_var, W_prob, W_unc, W_total):
    bands, diags, remb, remd = _build_mats(
        np.asarray(W_prob, np.float32),
        np.asarray(W_unc, np.float32),
        np.asarray(W_total, np.float32))
    in_maps = []
    for core in range(NCORES):
        b, half = divmod(core, 2)
        h0 = half * R
        var = np.zeros((2, R + 2, W + 2), np.float32)
        lo, hi = max(h0 - 1, 0), min(h0 + R + 1, H)
        var[0, lo - h0 + 1:hi - h0 + 1, 1:W + 1] = rgb_var[b, 0, lo:hi, :]
        var[1, lo - h0 + 1:hi - h0 + 1, 1:W + 1] = d_var[b, 0, lo:hi, :]
        in_maps.append({
            "rgb_s": np.ascontiguousarray(rgb[b, :, h0:h0 + R, :], np.float32),
            "d_s": np.ascontiguousarray(d[b, :, h0:h0 + R, :], np.float32),
            "var_s": var,
            "bands": bands, "diags": diags, "remb": remb, "remd": remd,
        })
    return in_maps


def run(trace=False, **inputs):
    if "nc" not in _CACHE:
        _CACHE["nc"] = _build_program()
    nc = _CACHE["nc"]
    in_maps = _shard_inputs(**inputs)
    res = run_bass_kernel_spmd(nc, in_maps, list(range(NCORES)), trace=trace)
    out = np.empty((B, C, H, W), np.float32)
    for core in range(NCORES):
        b, half = divmod(core, 2)
        out[b, :, half * R:(half + 1) * R, :] = res.results[core]["out_s"]
    return out, res


def kernel(**inputs):
    out, _ = run(trace=False, **inputs)
    return out
